# revision 10
# baseline (speedup 1.0000x reference)
"""Multi-head attention Trainium2 kernel (8 NeuronCores, head-parallel).

Sharding: core c handles heads {2c, 2c+1} for both batches.
 - Q/K/V projections column-parallel (each core: its 128 feature rows).
 - Attention per (b, head): scores computed TRANSPOSED ([t, s] layout) so the
   softmax denominator comes from a ones-column in the V-augmented ctx matmul.
 - Output projection row-parallel; partial outputs summed on host.
Outputs per core: attn_t [B, 2, S, S] ([b, h_local, t, s] = attn transposed) and
outp [B*S, D] (partial of out). Host: transpose attn_t, sum outp, add bo.
"""
import sys
import numpy as np

sys.path.insert(0, "/opt/trn_rl_repo")
sys.path.insert(0, "/opt/trn_rl_repo/concourse")

import concourse.bass as bass
import concourse.tile as tile
import concourse.mybir as mybir
from concourse import bacc
from concourse.bass_utils import run_bass_kernel_spmd
from concourse.masks import make_identity
from contextlib import ExitStack

B, S, D = 2, 2048, 1024
NH, DK = 16, 64
NC = 8
HPC = NH // NC          # heads per core = 2
F = HPC * DK            # feature rows per core = 128
f32 = mybir.dt.float32
f32r = mybir.dt.float32r
EXP = mybir.ActivationFunctionType.Exp
COPY = mybir.ActivationFunctionType.Copy
ts = bass.ts


def _body(nc, tc, st, io, loop_R=None):
    cpool = st.enter_context(tc.tile_pool(name="cpool", bufs=1))
    wpool = st.enter_context(tc.tile_pool(name="wpool", bufs=1))
    wstage = st.enter_context(tc.tile_pool(name="wstage", bufs=2))
    xpool = st.enter_context(tc.tile_pool(name="xpool", bufs=2))
    xtpool = st.enter_context(tc.tile_pool(name="xtpool", bufs=2))
    qkpool = st.enter_context(tc.tile_pool(name="qkpool", bufs=1))
    vpool = st.enter_context(tc.tile_pool(name="vpool", bufs=2))
    epool = st.enter_context(tc.tile_pool(name="epool", bufs=1))
    spool = st.enter_context(tc.tile_pool(name="spool", bufs=2))
    opool = st.enter_context(tc.tile_pool(name="opool", bufs=2))
    psA = st.enter_context(tc.tile_pool(name="psA", bufs=4, space="PSUM"))
    psB = st.enter_context(tc.tile_pool(name="psB", bufs=2, space="PSUM"))

    ident = cpool.tile([128, 128], f32)
    make_identity(nc, ident[:])
    ones_f = cpool.tile([128, 128], f32)
    nc.gpsimd.memset(ones_f[:], 1.0)
    ones = cpool.tile([128, 128], f32r)
    nc.vector.tensor_copy(ones[:], ones_f[:])
    identr = cpool.tile([128, 128], f32r)
    nc.vector.tensor_copy(identr[:], ident[:])

    # --- weight prep: wT_{q,k,v} [e=128, et, f=128], woT_h [f=64, o=1024] ---
    wT = {}
    for wname in ("wq", "wk", "wv"):
        wnat = wstage.tile([128, 1024], f32, name=f"wnat_{wname}", tag="wnat")
        nc.sync.dma_start(wnat[:], io[wname][:, :])  # [F=128, D]
        wt = wpool.tile([128, 8, 128], f32r, name=f"wT_{wname}")
        for etg in range(2):
            ps = psA.tile([128, 512], f32, name=f"psw_{wname}_{etg}", tag="ps512")
            for k in range(4):
                e = etg * 4 + k
                nc.tensor.transpose(ps[:, ts(k, 128)], wnat[:, ts(e, 128)], ident[:])
            if etg % 2 == 0:
                nc.scalar.copy(wt[:, ts(etg, 4)].rearrange("p a b -> p (a b)"), ps[:])
            else:
                nc.vector.tensor_copy(wt[:, ts(etg, 4)].rearrange("p a b -> p (a b)"), ps[:])
        wT[wname] = wt
    # wo slice is [D, F] = [1024, 128]; need woT_h [f 64, o 1024] per local head
    woT = []
    for h in range(HPC):
        wo_t = wpool.tile([64, 1024], f32r, name=f"woT_{h}")
        woT.append(wo_t)
    for ot in range(8):
        wos = wstage.tile([128, 128], f32, name=f"wos_{ot}", tag="wos")
        nc.sync.dma_start(wos[:], io["wo"][ts(ot, 128), :])
        ps = psA.tile([128, 512], f32, name=f"pswo_{ot}", tag="ps512")
        nc.tensor.transpose(ps[:, 0:128], wos[:], ident[:])
        for h in range(HPC):
            if ot % 2 == 0:
                nc.scalar.copy(woT[h][:, ts(ot, 128)], ps[ts(h, 64), 0:128])
            else:
                nc.vector.tensor_copy(woT[h][:, ts(ot, 128)], ps[ts(h, 64), 0:128])

    bias = {}
    for bname in ("bq", "bk", "bv"):
        bt = cpool.tile([128, 1], f32, name=f"bt_{bname}")
        nc.sync.dma_start(bt[:], io[bname][:, :])
        bias[bname] = bt

    def full_pass():
        for b in range(B):
            # ---------- P1: projections ----------
            QT = qkpool.tile([128, 2048], f32r, name=f"QT_{b}", tag="QT")
            KT = qkpool.tile([128, 2048], f32r, name=f"KT_{b}", tag="KT")
            vaug = [vpool.tile([128, 16, 65], f32r, name=f"vaug{h}_{b}", tag=f"vaug{h}")
                    for h in range(HPC)]
            for h in range(HPC):
                nc.vector.tensor_copy(vaug[h][:, :, 64:65], ones_f[:, 0:16])
            for tensor, wkey, bkey in (("k", "wk", "bk"), ("v", "wv", "bv"), ("q", "wq", "bq")):
                for sc in range(4):
                    r0 = b * 2048 + sc * 512
                    xnat = xpool.tile([128, 4, 1024], f32, name=f"xn_{tensor}_{b}_{sc}", tag="xnat")
                    nc.sync.dma_start(
                        xnat[:], io[tensor][r0:r0 + 512, :].rearrange("(a p) e -> p a e", p=128))
                    xt = xtpool.tile([128, 8, 512], f32r, name=f"xt_{tensor}_{b}_{sc}", tag="xt")
                    for et in range(8):
                        ps = psA.tile([128, 512], f32, name=f"pst_{et}", tag="ps512")
                        for stl in range(4):
                            nc.tensor.transpose(
                                ps[:, ts(stl, 128)], xnat[:, stl, ts(et, 128)], ident[:])
                        if et % 2 == 0:
                            nc.scalar.copy(xt[:, et, :], ps[:])
                        else:
                            nc.vector.tensor_copy(xt[:, et, :], ps[:])
                    psp = psA.tile([128, 512], f32, name="psp", tag="ps512")
                    for et in range(8):
                        nc.tensor.matmul(psp[:], wT[wkey][:, et, :], xt[:, et, :],
                                         start=(et == 0), stop=(et == 7))
                    if tensor == "k":
                        nc.vector.tensor_scalar_add(KT[:, ts(sc, 512)], psp[:], bias[bkey][:])
                    elif tensor == "q":
                        nc.vector.tensor_scalar_add(QT[:, ts(sc, 512)], psp[:], bias[bkey][:])
                    else:
                        vt = vpool.tile([128, 512], f32r, name="vt", tag="vt")
                        nc.vector.tensor_scalar_add(vt[:], psp[:], bias[bkey][:])
                        psv = psA.tile([128, 512], f32r, name="psv", tag="ps512")
                        for stl in range(4):
                            nc.tensor.transpose(psv[:, ts(stl, 128)], vt[:, ts(stl, 128)], identr[:])
                        for stl in range(4):
                            tt = sc * 4 + stl
                            nc.vector.tensor_copy(vaug[0][:, tt, 0:64], psv[:, stl * 128:stl * 128 + 64])
                            nc.scalar.copy(vaug[1][:, tt, 0:64], psv[:, stl * 128 + 64:stl * 128 + 128])

            # ---------- P2/P3 per s-chunk ----------
            for sc2 in range(4):
                ctxT = []
                for h in range(HPC):
                    expT = epool.tile([128, 16, 512], f32r, name=f"expT_{b}_{sc2}_{h}", tag="expT")
                    psc = psB.tile([128, 512], f32, name=f"psc_{h}", tag="psctx")
                    for tt in range(16):
                        pss = psA.tile([128, 512], f32, name="pss", tag="ps512")
                        nc.tensor.matmul(
                            pss[:], KT[ts(h, 64), ts(tt, 128)], QT[ts(h, 64), ts(sc2, 512)],
                            start=True, stop=True)
                        nc.scalar.activation(expT[:, tt, :], pss[:], EXP, scale=0.125)
                        nc.tensor.matmul(psc[0:65, :], vaug[h][:, tt, :], expT[:, tt, :],
                                         start=(tt == 0), stop=(tt == 15))
                    # reciprocal of sums (row 64), broadcast to 128 partitions via PE
                    rec = spool.tile([128, 512], f32r, name="rec", tag="rec")
                    with nc.allow_low_precision(reason="f32r recip for broadcast matmul"):
                        nc.vector.reciprocal(rec[64:65, :], psc[64:65, :])
                    psb = psA.tile([128, 512], f32, name="psb", tag="ps512")
                    nc.tensor.matmul(psb[:], ones[64:65, :], rec[64:65, :], start=True, stop=True)
                    rbc = spool.tile([128, 512], f32, name="rbc", tag="rbc")
                    nc.vector.tensor_copy(rbc[:], psb[:])
                    # normalize ctx^T -> SBUF (f32r) for out-proj
                    ct = spool.tile([64, 512], f32r, name=f"ct{h}", tag=f"ct{h}")
                    nc.vector.tensor_mul(ct[:], psc[0:64, :], rbc[0:64, :])
                    ctxT.append(ct)
                    # normalize attn tiles in place (f32r write keeps verifier happy) and DMA out
                    rbc_r = rbc[:].bitcast(f32r)
                    for g in range(4):
                        for tt in range(g * 4, g * 4 + 4):
                            eng = nc.vector if tt % 2 == 0 else nc.gpsimd
                            eng.tensor_mul(expT[:, tt, :], expT[:, tt, :], rbc_r)
                        dst = io["attn_t"][b, h].rearrange("(tt p) s -> p tt s", p=128)
                        nc.sync.dma_start(
                            dst[:, g * 4:g * 4 + 4, ts(sc2, 512)],
                            expT[:, g * 4:g * 4 + 4, :].bitcast(f32))
                # out projection for this s-chunk
                for stl in range(4):
                    outt = opool.tile([128, 1024], f32, name="outt", tag="outt")
                    for oc in range(2):
                        pso = psA.tile([128, 512], f32, name="pso", tag="ps512")
                        for h in range(HPC):
                            nc.tensor.matmul(pso[:], ctxT[h][:, ts(stl, 128)],
                                             woT[h][:, ts(oc, 512)],
                                             start=(h == 0), stop=(h == 1))
                        if oc == 0:
                            nc.scalar.copy(outt[:, ts(oc, 512)], pso[:])
                        else:
                            nc.vector.tensor_copy(outt[:, ts(oc, 512)], pso[:])
                    r0 = b * 2048 + sc2 * 512 + stl * 128
                    nc.sync.dma_start(io["outp"][r0:r0 + 128, :], outt[:])

    if loop_R is None:
        full_pass()
    else:
        with tc.For_i(0, loop_R, 1):
            full_pass()


def build_nc(loop_R=None):
    nc = bacc.Bacc("TRN2", target_bir_lowering=False, debug=False, num_devices=NC)
    io = {}
    for name in ("q", "k", "v"):
        io[name] = nc.dram_tensor(name, [B * S, D], f32, kind="ExternalInput").ap()
    for name in ("wq", "wk", "wv"):
        io[name] = nc.dram_tensor(name, [F, D], f32, kind="ExternalInput").ap()
    io["wo"] = nc.dram_tensor("wo", [D, F], f32, kind="ExternalInput").ap()
    for name in ("bq", "bk", "bv"):
        io[name] = nc.dram_tensor(name, [F, 1], f32, kind="ExternalInput").ap()
    io["attn_t"] = nc.dram_tensor("attn_t", [B, HPC, S, S], f32, kind="ExternalOutput").ap()
    io["outp"] = nc.dram_tensor("outp", [B * S, D], f32, kind="ExternalOutput").ap()
    with tile.TileContext(nc) as tc, ExitStack() as st:
        _body(nc, tc, st, io, loop_R=loop_R)
    nc.compile()
    return nc


def make_in_maps(q, k, v, wq, bq, wk, bk, wv, bv, wo, bo):
    qf = np.ascontiguousarray(q.reshape(B * S, D), dtype=np.float32)
    kf = np.ascontiguousarray(k.reshape(B * S, D), dtype=np.float32)
    vf = np.ascontiguousarray(v.reshape(B * S, D), dtype=np.float32)
    in_maps = []
    for c in range(NC):
        fsl = slice(c * F, (c + 1) * F)
        in_maps.append({
            "q": qf, "k": kf, "v": vf,
            "wq": np.ascontiguousarray(wq[fsl, :], dtype=np.float32),
            "wk": np.ascontiguousarray(wk[fsl, :], dtype=np.float32),
            "wv": np.ascontiguousarray(wv[fsl, :], dtype=np.float32),
            "wo": np.ascontiguousarray(wo[:, fsl], dtype=np.float32),
            "bq": np.ascontiguousarray(bq[fsl].reshape(F, 1), dtype=np.float32),
            "bk": np.ascontiguousarray(bk[fsl].reshape(F, 1), dtype=np.float32),
            "bv": np.ascontiguousarray(bv[fsl].reshape(F, 1), dtype=np.float32),
        })
    return in_maps


def gather(results, bo):
    out = np.zeros((B * S, D), dtype=np.float32)
    attn = np.empty((B, NH, S, S), dtype=np.float32)
    for c in range(NC):
        out += results[c]["outp"]
        at = results[c]["attn_t"]  # [B, HPC, S(t), S(s)]
        for h in range(HPC):
            attn[:, c * HPC + h] = at[:, h].transpose(0, 2, 1)
    out = out + np.asarray(bo, dtype=np.float32)[None, :]
    return out.reshape(B, S, D), attn


_NC_CACHE = {}


def kernel(q, k, v, wq, bq, wk, bk, wv, bv, wo, bo):
    if "nc" not in _NC_CACHE:
        _NC_CACHE["nc"] = build_nc()
    nc = _NC_CACHE["nc"]
    in_maps = make_in_maps(q, k, v, wq, bq, wk, bk, wv, bv, wo, bo)
    res = run_bass_kernel_spmd(nc, in_maps, core_ids=list(range(NC)), trace=False)
    return gather(res.results, bo)
